# revision 2
# baseline (speedup 1.0000x reference)
"""DiffAttention Trainium2 kernel, v2: software-pipelined units.

Math (per batch b, head h):
  q,k split into two streams of D=64; v has E=128 channels.
  attn_s = softmax_causal(q_s k_s^T / 8) @ v            (s = 1,2)
  lam    = exp(lq1.lk1) - exp(lq2.lk2) + 0.8            (host scalar)
  x      = attn1 - lam*attn2
  out    = 0.2 * w * x * rsqrt(mean_e(x^2) + eps)

v2 strategy (vs v1):
  - Work unit = (pair, q-block, stream). Exp'd probabilities P^T for a
    whole unit are RETAINED in SBUF, decoupling the PV matmuls from the
    S->exp chain. PV of unit n-1 is interleaved between the S-matmul
    chunks of unit n so the PE never drains (the TRN2 tensor engine
    only reaches 2.4GHz after ~3us of gapless execution; any stall
    drops it to 1.2GHz).
  - ~25% of the exp work (early, non-diagonal k-chunks) moves from the
    ACT engine to the DVE via a one-op fast-exp: bf16 bits of exp(x)
    are approximated by int16(round(A*x + B)) (Schraudolph in bf16).
  - PSUM: 4 banks st double-buffer (2 tags x 2 banks), 4 banks PV
    accumulators. u results are copied to SBUF each unit (hidden
    behind the next unit's S burst); the epilogue runs SBUF-only.
  - Softmax normalization deferred via RMSNorm scale-invariance:
      x ∝ U1*den2 - lam*U2*den1  (U_s unnormalized, den_s row sums,
    dens obtained free via a leading ones-column in V).
  - rsqrt on DVE (quake seed + 3 Newton iters), exp-only ACT tables.
"""

from contextlib import ExitStack

import numpy as np

import concourse.bass as bass
import concourse.mybir as mybir
from concourse import bacc
from concourse._compat import axon_active
from concourse.bass import MemorySpace
from concourse.bass_utils import run_bass_kernel_spmd
from concourse.tile import TileContext

F32 = mybir.dt.float32
BF16 = mybir.dt.bfloat16
I16 = mybir.dt.int16
I32 = mybir.dt.int32
AF = mybir.ActivationFunctionType
ALU = mybir.AluOpType

B, L, H, D = 2, 2048, 8, 64
E = 2 * D               # 128 v-channels per head
NP = 128                # SBUF partitions
PAIRS = 2               # (b,h) pairs per core
N_CORES = 8
QB = 512                # q columns per block (4 tiles of 128)
NQB = L // QB           # 4
CHUNK = 2               # k-tiles exp'd per op
NKT = L // NP           # 16 k tiles
LAMBDA_INIT = 0.8
EPS = 1e-5
OUT_SCALE = 1.0 - LAMBDA_INIT  # 0.2
SM_SCALE = 1.0 / 8.0

# DVE fast-exp (bf16 Schraudolph): bf16_bits(exp(x)) ~ int16(A*x + B).
FE_C = 0.04367
FE_A = 128.0 / np.log(2.0) * SM_SCALE      # folds the softmax scale
FE_B = 128.0 * (127.0 - FE_C)
# chunks 0..NOFF[qb]-1 of each (qb, s) unit run on DVE instead of ACT
NOFF = (0, 1, 2, 2)


def _build_program(w_is_ones: bool, repeat: int = 1, skip: frozenset = frozenset(),
                   noff=NOFF, interleave: bool = True,
                   off_late: bool = False, upasses: int = 2,
                   st_tags: int = 3, staircase: bool = True,
                   fine_batch: bool = True,
                   tail_reorder: bool = True, epi_act: bool = False,
                   epi_pool: bool = False, lag2: bool = False,
                   ucopy_act: bool = True) -> bass.Bass:
    nc = bacc.Bacc(
        "TRN2",
        target_bir_lowering=False,
        debug=not axon_active(),
        enable_asserts=False,
        num_devices=N_CORES,
    )
    qt_d = nc.declare_dram_parameter("qt", [PAIRS, NP, L], BF16, isOutput=False)
    kt_d = nc.declare_dram_parameter("kt", [PAIRS, NP, L], BF16, isOutput=False)
    vv_d = nc.declare_dram_parameter("vv", [PAIRS, L, E + 1], BF16, isOutput=False)
    lam_d = nc.declare_dram_parameter("lam", [NP, 1], F32, isOutput=False)
    if not w_is_ones:
        wb_d = nc.declare_dram_parameter("wb", [NP, E], F32, isOutput=False)
    out_d = nc.declare_dram_parameter("out", [PAIRS, L, E], F32, isOutput=True)

    with TileContext(nc) as tc, ExitStack() as ctx:
        const = ctx.enter_context(tc.tile_pool(name="const", bufs=1))
        io = ctx.enter_context(tc.tile_pool(name="io", bufs=2))
        ptp = ctx.enter_context(tc.tile_pool(name="ptp", bufs=1))
        usb = ctx.enter_context(tc.tile_pool(name="usb", bufs=2))
        ep = ctx.enter_context(tc.tile_pool(name="ep", bufs=2))
        xp = ctx.enter_context(tc.tile_pool(name="xp", bufs=2))
        stp = ctx.enter_context(
            tc.tile_pool(name="stp", bufs=1, space=MemorySpace.PSUM)
        )
        up = ctx.enter_context(tc.tile_pool(name="up", bufs=1, space=MemorySpace.PSUM))

        lam_sb = const.tile([NP, 1], F32)
        nc.sync.dma_start(lam_sb[:], lam_d[:])
        magic = const.tile([NP, 1], I32)
        nc.gpsimd.memset(magic[:], 0x5F3759DF)
        if not w_is_ones:
            wb_sb = const.tile([NP, E], F32)
            nc.sync.dma_start(wb_sb[:], wb_d[:])

        st_par = [0]

        def emit_s_chunk(ud, c):
            """S matmuls + exp (+ diag masks) for chunk c of unit ud."""
            cn = min(CHUNK, ud["nki"] - c * CHUNK)
            kc = c * CHUNK
            s, qb = ud["s"], ud["qb"]
            sp = slice(s * D, (s + 1) * D)
            st_par[0] = (st_par[0] + 1) % st_tags
            st = stp.tile([NP, CHUNK, QB], F32,
                          tag=f"st{st_par[0]}", name=f"st{st_par[0]}")
            offs = []
            for j in range(cn):
                ki = kc + j
                offs.append(max(0, ki - 4 * qb) * NP if staircase else 0)
            if "s" not in skip:
                for j in range(cn):
                    ki = kc + j
                    nc.tensor.matmul(
                        st[:, j, offs[j]:],
                        ud["kt"][sp, ki * NP:(ki + 1) * NP],
                        ud["qt"][sp, qb * QB + offs[j]:(qb + 1) * QB],
                        start=True, stop=True,
                    )
            pt = ud["pt"]
            if off_late:
                # offloaded chunks sit just below the diagonal block
                use_dve = 2 * qb - noff[qb] <= c < 2 * qb
            else:
                use_dve = c < noff[qb]
            if "exp" in skip:
                nc.scalar.activation(pt[:, kc, 0:1], st[:, 0, 0:1], AF.Exp,
                                     scale=SM_SCALE)
            elif use_dve:
                nc.vector.tensor_scalar(
                    pt[:, kc:kc + cn, :].bitcast(I16), st[:, :cn, :],
                    float(FE_A), float(FE_B), ALU.mult, ALU.add,
                )
            elif any(offs):
                for j in range(cn):
                    nc.scalar.activation(pt[:, kc + j, offs[j]:],
                                         st[:, j, offs[j]:],
                                         AF.Exp, scale=SM_SCALE)
            else:
                nc.scalar.activation(pt[:, kc:kc + cn, :], st[:, :cn, :],
                                     AF.Exp, scale=SM_SCALE)
            if "mask" not in skip:
                for j in range(cn):
                    ki = kc + j
                    qi = ki - 4 * qb
                    if 0 <= qi < 4:  # diagonal tile: zero above diagonal
                        sl = pt[:, ki, qi * NP:(qi + 1) * NP]
                        nc.gpsimd.affine_select(
                            sl, sl,
                            pattern=[[1, NP]],
                            compare_op=ALU.is_ge,
                            fill=0.0,
                            base=0,
                            channel_multiplier=-1,
                        )

        QPP = 4 // upasses  # qi per PSUM pass

        def build_stream(ud):
            """PV matmuls in upasses qi-passes + per-pass u-copies.
            Allocates the pass PSUM tiles; call once, right before emitting."""
            qb = ud["qb"]
            items = []
            for pidx in range(upasses):
                u_ps = up.tile([NP, QPP, QB], F32, tag="u", name="u")
                qlo = pidx * QPP
                for ki in range(ud["nki"]):
                    for qi in range(max(qlo, ki - 4 * qb), qlo + QPP):
                        qt_g = 4 * qb + qi
                        items.append(("pv", u_ps, qi - qlo, ki, qi,
                                      ki == 0, ki == qt_g))
                items.append(("ucopy", u_ps, qlo))
            return items

        def emit_stream(ud, items):
            if "pv" in skip:
                return
            for it in items:
                if it[0] == "pv":
                    _, u_ps, ql, ki, qi, start, stop = it
                    nc.tensor.matmul(
                        u_ps[:, ql, 0:E + 1],
                        ud["pt"][:, ki, qi * NP:(qi + 1) * NP],
                        ud["vv"][:, ki, :],
                        start=start, stop=stop,
                    )
                else:
                    _, u_ps, qlo = it
                    if "epi" not in skip:
                        if ucopy_act:
                            nc.scalar.copy(
                                ud["u_sb"][:, qlo:qlo + QPP, :],
                                u_ps[:, :, 0:E + 1])
                        else:
                            nc.vector.tensor_copy(
                                ud["u_sb"][:, qlo:qlo + QPP, :],
                                u_ps[:, :, 0:E + 1])

        def finish_unit(ud):
            """u-copy; epilogue after s=1; finale after qb=3,s=1."""
            if "pv" in skip or "epi" in skip:
                if ud["last"]:
                    o_sb = ud["o_sb"]
                    nc.gpsimd.memset(o_sb[:], 0.0)
                    nc.sync.dma_start(
                        out_d[ud["p"]].rearrange("(t q) e -> q t e", q=NP),
                        o_sb[:],
                    )
                return
            if ud["s"] == 0:
                return
            # ---- epilogue for (p, qb): combine streams -------------------
            qb, p = ud["qb"], ud["p"]
            u0, u1 = ud["peer"]["u_sb"], ud["u_sb"]
            x_all, ms_all = ud["x_all"], ud["ms_all"]
            d1l = ep.tile([NP, 4], F32, tag="d1l")  # lam * den1
            nc.vector.tensor_scalar(
                d1l[:], u0[:, :, 0], lam_sb[:, 0:1], None, ALU.mult
            )
            # eps applies to normalized x -> deferred ms needs eps*(d1*d2)^2
            dd = ep.tile([NP, 4], F32, tag="dd")  # sqrt(eps)*den1*den2
            nc.vector.scalar_tensor_tensor(
                dd[:], u0[:, :, 0], float(np.sqrt(EPS)), u1[:, :, 0],
                ALU.mult, ALU.mult,
            )
            edd = ep.tile([NP, 4], F32, tag="edd")  # eps*(den1*den2)^2
            nc.vector.tensor_tensor(edd[:], dd[:], dd[:], ALU.mult)
            t2 = ep.tile([NP, 4, E], F32, tag="t2")  # lam*den1*U2
            eng_t2 = nc.gpsimd if epi_pool else nc.vector
            eng_t2.tensor_tensor(
                t2[:], u1[:, :, 1:E + 1],
                d1l[:].unsqueeze(2).broadcast_to([NP, 4, E]),
                ALU.mult,
            )
            for qi in range(4):
                qt_g = 4 * qb + qi
                nc.vector.scalar_tensor_tensor(
                    x_all[:, qt_g, :],
                    u0[:, qi, 1:E + 1],
                    u1[:, qi, 0:1],
                    t2[:, qi, :],
                    ALU.mult, ALU.subtract,
                )
                if epi_act:
                    # ms = mean(x^2) on ACT: Square(x/sqrt(E)) + free-axis accum
                    nc.scalar.activation(
                        ud["o_sb"][:, qt_g, :], x_all[:, qt_g, :], AF.Square,
                        scale=float(E) ** -0.5,
                        accum_out=ms_all[:, qt_g:qt_g + 1],
                    )
                else:
                    xsq = ep.tile([NP, E], F32, tag="xsq")
                    nc.vector.scalar_tensor_tensor(
                        xsq[:], x_all[:, qt_g, :], 1.0 / E, x_all[:, qt_g, :],
                        ALU.mult, ALU.mult,
                    )
                    nc.vector.reduce_sum(
                        ms_all[:, qt_g:qt_g + 1], xsq[:],
                        axis=mybir.AxisListType.X,
                    )
            nc.vector.tensor_tensor(
                ms_all[:, 4 * qb:4 * qb + 4],
                ms_all[:, 4 * qb:4 * qb + 4],
                edd[:], ALU.add,
            )
            if not ud["last"]:
                return
            # ---- finale for pair p: rs = 0.2*rsqrt(ms); out --------------
            sh = ep.tile([NP, NKT], I32, tag="sh")
            nc.vector.tensor_scalar(
                sh[:], ms_all[:].bitcast(I32), 1, None, ALU.logical_shift_right
            )
            y = ep.tile([NP, NKT], F32, tag="y")
            nc.vector.tensor_tensor(
                y[:].bitcast(I32),
                magic[:].broadcast_to([NP, NKT]).bitcast(I32),
                sh[:], ALU.subtract,
            )
            mh = ep.tile([NP, NKT], F32, tag="mh")  # 0.5*ms
            nc.vector.tensor_scalar(mh[:], ms_all[:], 0.5, None, ALU.mult)
            for it in range(3):
                yy = ep.tile([NP, NKT], F32, tag="yy", name="yy")
                nc.vector.tensor_tensor(yy[:], y[:], y[:], ALU.mult)
                nc.vector.tensor_tensor(yy[:], yy[:], mh[:], ALU.mult)
                fin = -OUT_SCALE if it == 2 else -1.0
                nc.vector.tensor_scalar(
                    yy[:], yy[:], 1.5, fin, ALU.subtract, ALU.mult
                )
                yn = ep.tile([NP, NKT], F32, tag="yn", name="yn")
                nc.vector.tensor_tensor(yn[:], y[:], yy[:], ALU.mult)
                y = yn
            o_sb = ud["o_sb"]
            if epi_act:
                for t in range(NKT):
                    nc.scalar.activation(o_sb[:, t, :], x_all[:, t, :],
                                         AF.Copy, scale=y[:, t:t + 1])
            else:
                nc.vector.tensor_tensor(
                    o_sb[:], x_all[:],
                    y[:].unsqueeze(2).broadcast_to([NP, NKT, E]),
                    ALU.mult,
                )
            if not w_is_ones:
                nc.vector.tensor_tensor(
                    o_sb[:], o_sb[:],
                    wb_sb[:].unsqueeze(1).broadcast_to([NP, NKT, E]),
                    ALU.mult,
                )
            nc.sync.dma_start(
                out_d[p].rearrange("(t q) e -> q t e", q=NP), o_sb[:]
            )

        rep_ctx = tc.For_i(0, repeat, 1) if repeat > 1 else None
        if rep_ctx is not None:
            ctx.enter_context(rep_ctx)

        if "compute" in skip:  # DMA-only floor probe
            for p in range(PAIRS):
                qt_sb = io.tile([NP, L], BF16, tag="qt", name="qt")
                kt_sb = io.tile([NP, L], BF16, tag="kt", name="kt")
                vv_sb = io.tile([NP, NKT, E + 1], BF16, tag="vv", name="vv")
                nc.sync.dma_start(qt_sb[:], qt_d[p])
                nc.sync.dma_start(kt_sb[:], kt_d[p])
                nc.sync.dma_start(
                    vv_sb[:], vv_d[p].rearrange("(t k) e -> k t e", k=NP))
                o_sb = xp.tile([NP, NKT, E], F32, tag="o", name="o")
                nc.gpsimd.memset(o_sb[:, 0, 0:1], 0.0)
                nc.vector.tensor_tensor(
                    o_sb[:, 0, 0:1], qt_sb[:, 0:2].bitcast(F32),
                    kt_sb[:, 0:2].bitcast(F32), ALU.mult)
                nc.vector.tensor_copy(o_sb[:, 0, 2:4], vv_sb[:, 0, 0:2])
                nc.sync.dma_start(
                    out_d[p].rearrange("(t q) e -> q t e", q=NP), o_sb[:])
            units = []
        else:
            units = []
            for p in range(PAIRS):
                qbs = list(range(NQB))
                if tail_reorder and p == PAIRS - 1:
                    qbs = qbs[::-1]
                for i_qb, qb in enumerate(qbs):
                    for s in range(2):
                        units.append((p, qb, s, i_qb == 0 and s == 0,
                                      i_qb == NQB - 1 and s == 1))

        pending = None  # previous unit: PV not yet emitted
        pair_state = {}
        pvq = []  # rolling queue of (unit, stream-item, is_last)

        def drain_pvq(n):
            for _ in range(n):
                if not pvq:
                    return
                ud_i, it, last = pvq.pop(0)
                emit_stream(ud_i, [it])
                if last:
                    finish_unit(ud_i)
        for p, qb, s, first_of_pair, last_of_pair in units:
                    if first_of_pair:
                        qt_sb = io.tile([NP, L], BF16, tag="qt", name="qt")
                        kt_sb = io.tile([NP, L], BF16, tag="kt", name="kt")
                        vv_sb = io.tile([NP, NKT, E + 1], BF16, tag="vv",
                                        name="vv")
                        nc.sync.dma_start(qt_sb[:], qt_d[p])
                        nc.sync.dma_start(kt_sb[:], kt_d[p])
                        nc.sync.dma_start(
                            vv_sb[:],
                            vv_d[p].rearrange("(t k) e -> k t e", k=NP))
                        x_all = xp.tile([NP, NKT, E], F32, tag="x", name="x")
                        ms_all = xp.tile([NP, NKT], F32, tag="ms", name="ms")
                        o_sb = xp.tile([NP, NKT, E], F32, tag="o", name="o")
                        pair_state = dict(qt=qt_sb, kt=kt_sb, vv=vv_sb,
                                          x_all=x_all, ms_all=ms_all, o_sb=o_sb)
                    nki = 4 * qb + 4
                    ud = dict(p=p, qb=qb, s=s, nki=nki, last=last_of_pair,
                              **pair_state)
                    ud["pt"] = ptp.tile([NP, nki, QB], BF16, tag=f"pt{s}",
                                        name=f"pt{s}")
                    if "pv" not in skip and "epi" not in skip:
                        ud["u_sb"] = usb.tile([NP, 4, E + 1], F32,
                                              tag=f"u{s}", name=f"u{s}")
                    if s == 1:
                        ud["peer"] = prev_unit  # the s=0 unit of same (p, qb)

                    pushed = 0
                    if pending is not None:
                        st_items = build_stream(pending)
                        for idx, it in enumerate(st_items):
                            pvq.append((pending, it,
                                        idx == len(st_items) - 1))
                        pushed = len(st_items)
                    nchunks = nki // CHUNK
                    if interleave:
                        keep = pushed // 2 if lag2 else 0
                        target = max(0, len(pvq) - keep)
                        nb = nchunks if fine_batch else max(1, nchunks - 1)
                        bsz = (target + nb - 1) // nb if target else 0
                        done = 0
                        for c in range(nchunks):
                            emit_s_chunk(ud, c)
                            if (fine_batch or c >= 1) and done < target:
                                take = min(bsz, target - done)
                                drain_pvq(take)
                                done += take
                        if done < target:
                            drain_pvq(target - done)
                    else:
                        drain_pvq(len(pvq))
                        for c in range(nchunks):
                            emit_s_chunk(ud, c)
                    prev_unit = ud
                    pending = ud
        # drain
        if pending is not None:
            st_items = build_stream(pending)
            for idx, it in enumerate(st_items):
                pvq.append((pending, it, idx == len(st_items) - 1))
        drain_pvq(len(pvq))

    nc.compile()
    return nc


_PROGRAM_CACHE: dict = {}


def _get_program(w_is_ones: bool, repeat: int = 1) -> bass.Bass:
    key = (w_is_ones, repeat)
    if key not in _PROGRAM_CACHE:
        _PROGRAM_CACHE[key] = _build_program(w_is_ones, repeat)
    return _PROGRAM_CACHE[key]


def make_in_maps(query, key, value, lambda_q1, lambda_k1, lambda_q2, lambda_k2,
                 sub_norm_w):
    """Host-side shard/pack. Returns (in_maps, w_is_ones)."""
    query = np.asarray(query, dtype=np.float32)
    key = np.asarray(key, dtype=np.float32)
    value = np.asarray(value, dtype=np.float32)
    lam = float(
        np.exp(np.sum(np.float64(lambda_q1) * np.float64(lambda_k1)))
        - np.exp(np.sum(np.float64(lambda_q2) * np.float64(lambda_k2)))
        + LAMBDA_INIT
    )
    w = np.asarray(sub_norm_w, dtype=np.float32)
    w_is_ones = bool(np.all(w == 1.0))

    import ml_dtypes

    bf16 = ml_dtypes.bfloat16
    q5 = query.reshape(B, L, H, 2 * D)
    k5 = key.reshape(B, L, H, 2 * D)
    v4 = value.reshape(B, L, H, E)
    lam_arr = np.full((NP, 1), lam, dtype=np.float32)
    wb = np.broadcast_to(w[None, :], (NP, E)).copy() if not w_is_ones else None

    in_maps = []
    for c in range(N_CORES):
        qt = np.empty((PAIRS, NP, L), dtype=bf16)
        kt = np.empty((PAIRS, NP, L), dtype=bf16)
        vv = np.empty((PAIRS, L, E + 1), dtype=bf16)
        for p in range(PAIRS):
            f = c * PAIRS + p
            b, h = divmod(f, H)
            qt[p] = q5[b, :, h].T.astype(bf16)
            kt[p] = k5[b, :, h].T.astype(bf16)
            vv[p, :, 0] = 1.0
            vv[p, :, 1:] = v4[b, :, h].astype(bf16)
        m = {"qt": qt, "kt": kt, "vv": vv, "lam": lam_arr}
        if not w_is_ones:
            m["wb"] = wb
        in_maps.append(m)
    return in_maps, w_is_ones


def assemble_output(results) -> np.ndarray:
    out = np.empty((B, L, H * E), dtype=np.float32)
    for c in range(N_CORES):
        o = results[c]["out"]
        for p in range(PAIRS):
            f = c * PAIRS + p
            b, h = divmod(f, H)
            out[b, :, h * E:(h + 1) * E] = o[p]
    return out


def kernel(query, key, value, lambda_q1, lambda_k1, lambda_q2, lambda_k2,
           sub_norm_w, **_unused):
    in_maps, w_is_ones = make_in_maps(
        query, key, value, lambda_q1, lambda_k1, lambda_q2, lambda_k2, sub_norm_w
    )
    nc = _get_program(w_is_ones)
    res = run_bass_kernel_spmd(nc, in_maps, core_ids=list(range(N_CORES)))
    return assemble_output(res.results)


# revision 5
# speedup vs baseline: 1.0410x; 1.0410x over previous
"""DiffAttention Trainium2 kernel, v2: software-pipelined units.

Math (per batch b, head h):
  q,k split into two streams of D=64; v has E=128 channels.
  attn_s = softmax_causal(q_s k_s^T / 8) @ v            (s = 1,2)
  lam    = exp(lq1.lk1) - exp(lq2.lk2) + 0.8            (host scalar)
  x      = attn1 - lam*attn2
  out    = 0.2 * w * x * rsqrt(mean_e(x^2) + eps)

v2 strategy (vs v1):
  - Work unit = (pair, q-block, stream). Exp'd probabilities P^T for a
    whole unit are RETAINED in SBUF, decoupling the PV matmuls from the
    S->exp chain. PV of unit n-1 is interleaved between the S-matmul
    chunks of unit n so the PE never drains (the TRN2 tensor engine
    only reaches 2.4GHz after ~3us of gapless execution; any stall
    drops it to 1.2GHz).
  - ~25% of the exp work (early, non-diagonal k-chunks) moves from the
    ACT engine to the DVE via a one-op fast-exp: bf16 bits of exp(x)
    are approximated by int16(round(A*x + B)) (Schraudolph in bf16).
  - PSUM: 4 banks st double-buffer (2 tags x 2 banks), 4 banks PV
    accumulators. u results are copied to SBUF each unit (hidden
    behind the next unit's S burst); the epilogue runs SBUF-only.
  - Softmax normalization deferred via RMSNorm scale-invariance:
      x ∝ U1*den2 - lam*U2*den1  (U_s unnormalized, den_s row sums,
    dens obtained free via a leading ones-column in V).
  - rsqrt on DVE (quake seed + 3 Newton iters), exp-only ACT tables.
"""

from contextlib import ExitStack

import numpy as np

import concourse.bass as bass
import concourse.mybir as mybir
from concourse import bacc
from concourse._compat import axon_active
from concourse.bass import MemorySpace
from concourse.bass_utils import run_bass_kernel_spmd
from concourse.tile import TileContext

F32 = mybir.dt.float32
BF16 = mybir.dt.bfloat16
I16 = mybir.dt.int16
I32 = mybir.dt.int32
AF = mybir.ActivationFunctionType
ALU = mybir.AluOpType

B, L, H, D = 2, 2048, 8, 64
E = 2 * D               # 128 v-channels per head
NP = 128                # SBUF partitions
PAIRS = 2               # (b,h) pairs per core
N_CORES = 8
QB = 512                # q columns per block (4 tiles of 128)
NQB = L // QB           # 4
CHUNK = 2               # k-tiles exp'd per op
NKT = L // NP           # 16 k tiles
LAMBDA_INIT = 0.8
EPS = 1e-5
OUT_SCALE = 1.0 - LAMBDA_INIT  # 0.2
SM_SCALE = 1.0 / 8.0

# DVE fast-exp (bf16 Schraudolph): bf16_bits(exp(x)) ~ int16(A*x + B).
FE_C = 0.04367
FE_A = 128.0 / np.log(2.0) * SM_SCALE      # folds the softmax scale
FE_B = 128.0 * (127.0 - FE_C)
# chunks 0..NOFF[qb]-1 of each (qb, s) unit run on DVE instead of ACT
NOFF = (0, 1, 2, 2)


def _build_program(w_is_ones: bool, repeat: int = 1, skip: frozenset = frozenset(),
                   noff=NOFF, interleave: bool = True,
                   off_late: bool = False, upasses: int = 2,
                   st_tags: int = 3, staircase: bool = True,
                   fine_batch: bool = True,
                   tail_reorder: bool = True, epi_act: bool = False,
                   epi_pool: bool = False, lag2: bool = False,
                   ucopy_act: bool = True, ucopy_split: bool = False,
                   qb_finale: bool = True, odma_pool: bool = False,
                   deep_bufs: bool = False,
                   pv_first_slot: bool = False) -> bass.Bass:
    nc = bacc.Bacc(
        "TRN2",
        target_bir_lowering=False,
        debug=not axon_active(),
        enable_asserts=False,
        num_devices=N_CORES,
    )
    qt_d = nc.declare_dram_parameter("qt", [PAIRS, NP, L], BF16, isOutput=False)
    kt_d = nc.declare_dram_parameter("kt", [PAIRS, NP, L], BF16, isOutput=False)
    vv_d = nc.declare_dram_parameter("vv", [PAIRS, L, E + 1], BF16, isOutput=False)
    lam_d = nc.declare_dram_parameter("lam", [NP, 1], F32, isOutput=False)
    if not w_is_ones:
        wb_d = nc.declare_dram_parameter("wb", [NP, E], F32, isOutput=False)
    out_d = nc.declare_dram_parameter("out", [PAIRS, L, E], F32, isOutput=True)

    with TileContext(nc) as tc, ExitStack() as ctx:
        const = ctx.enter_context(tc.tile_pool(name="const", bufs=1))
        io = ctx.enter_context(tc.tile_pool(name="io", bufs=2))
        ptp = ctx.enter_context(tc.tile_pool(name="ptp", bufs=1))
        nbuf = 3 if deep_bufs else 2
        usb = ctx.enter_context(tc.tile_pool(name="usb", bufs=nbuf))
        ep = ctx.enter_context(tc.tile_pool(name="ep", bufs=nbuf))
        xp = ctx.enter_context(tc.tile_pool(name="xp", bufs=2))
        stp = ctx.enter_context(
            tc.tile_pool(name="stp", bufs=1, space=MemorySpace.PSUM)
        )
        up = ctx.enter_context(tc.tile_pool(name="up", bufs=1, space=MemorySpace.PSUM))

        lam_sb = const.tile([NP, 1], F32)
        nc.sync.dma_start(lam_sb[:], lam_d[:])
        magic = const.tile([NP, 1], I32)
        nc.gpsimd.memset(magic[:], 0x5F3759DF)
        if not w_is_ones:
            wb_sb = const.tile([NP, E], F32)
            nc.sync.dma_start(wb_sb[:], wb_d[:])

        st_par = [0]

        def emit_s_chunk(ud, c):
            """S matmuls + exp (+ diag masks) for chunk c of unit ud."""
            cn = min(CHUNK, ud["nki"] - c * CHUNK)
            kc = c * CHUNK
            s, qb = ud["s"], ud["qb"]
            sp = slice(s * D, (s + 1) * D)
            st_par[0] = (st_par[0] + 1) % st_tags
            st = stp.tile([NP, CHUNK, QB], F32,
                          tag=f"st{st_par[0]}", name=f"st{st_par[0]}")
            offs = []
            for j in range(cn):
                ki = kc + j
                offs.append(max(0, ki - 4 * qb) * NP if staircase else 0)
            if "s" not in skip:
                for j in range(cn):
                    ki = kc + j
                    nc.tensor.matmul(
                        st[:, j, offs[j]:],
                        ud["kt"][sp, ki * NP:(ki + 1) * NP],
                        ud["qt"][sp, qb * QB + offs[j]:(qb + 1) * QB],
                        start=True, stop=True,
                    )
            pt = ud["pt"]
            if off_late:
                # offloaded chunks sit just below the diagonal block
                use_dve = 2 * qb - noff[qb] <= c < 2 * qb
            else:
                use_dve = c < noff[qb]
            if "exp" in skip:
                nc.scalar.activation(pt[:, kc, 0:1], st[:, 0, 0:1], AF.Exp,
                                     scale=SM_SCALE)
            elif use_dve:
                nc.vector.tensor_scalar(
                    pt[:, kc:kc + cn, :].bitcast(I16), st[:, :cn, :],
                    float(FE_A), float(FE_B), ALU.mult, ALU.add,
                )
            elif any(offs):
                for j in range(cn):
                    nc.scalar.activation(pt[:, kc + j, offs[j]:],
                                         st[:, j, offs[j]:],
                                         AF.Exp, scale=SM_SCALE)
            else:
                nc.scalar.activation(pt[:, kc:kc + cn, :], st[:, :cn, :],
                                     AF.Exp, scale=SM_SCALE)
            if "mask" not in skip:
                for j in range(cn):
                    ki = kc + j
                    qi = ki - 4 * qb
                    if 0 <= qi < 4:  # diagonal tile: zero above diagonal
                        sl = pt[:, ki, qi * NP:(qi + 1) * NP]
                        nc.gpsimd.affine_select(
                            sl, sl,
                            pattern=[[1, NP]],
                            compare_op=ALU.is_ge,
                            fill=0.0,
                            base=0,
                            channel_multiplier=-1,
                        )

        QPP = 4 // upasses  # qi per PSUM pass

        def build_stream(ud):
            """PV matmuls in upasses qi-passes + per-pass u-copies.
            Allocates the pass PSUM tiles; call once, right before emitting."""
            qb = ud["qb"]
            items = []
            for pidx in range(upasses):
                u_ps = up.tile([NP, QPP, QB], F32, tag="u", name="u")
                qlo = pidx * QPP
                for ki in range(ud["nki"]):
                    for qi in range(max(qlo, ki - 4 * qb), qlo + QPP):
                        qt_g = 4 * qb + qi
                        items.append(("pv", u_ps, qi - qlo, ki, qi,
                                      ki == 0, ki == qt_g))
                items.append(("ucopy", u_ps, qlo))
            return items

        def emit_stream(ud, items):
            if "pv" in skip:
                return
            for it in items:
                if it[0] == "pv":
                    _, u_ps, ql, ki, qi, start, stop = it
                    nc.tensor.matmul(
                        u_ps[:, ql, 0:E + 1],
                        ud["pt"][:, ki, qi * NP:(qi + 1) * NP],
                        ud["vv"][:, ki, :],
                        start=start, stop=stop,
                    )
                else:
                    _, u_ps, qlo = it
                    if "epi" not in skip:
                        use_act = ucopy_act and (not ucopy_split or qlo == 0)
                        if use_act:
                            nc.scalar.copy(
                                ud["u_sb"][:, qlo:qlo + QPP, :],
                                u_ps[:, :, 0:E + 1])
                        else:
                            nc.vector.tensor_copy(
                                ud["u_sb"][:, qlo:qlo + QPP, :],
                                u_ps[:, :, 0:E + 1])

        def finish_unit(ud):
            """u-copy; epilogue after s=1; finale after qb=3,s=1."""
            if "pv" in skip or "epi" in skip:
                if ud["last"]:
                    o_sb = ud["o_sb"]
                    nc.gpsimd.memset(o_sb[:], 0.0)
                    nc.sync.dma_start(
                        out_d[ud["p"]].rearrange("(t q) e -> q t e", q=NP),
                        o_sb[:],
                    )
                return
            if ud["s"] == 0:
                return
            # ---- epilogue for (p, qb): combine streams -------------------
            qb, p = ud["qb"], ud["p"]
            u0, u1 = ud["peer"]["u_sb"], ud["u_sb"]
            x_all, ms_all = ud["x_all"], ud["ms_all"]
            d1l = ep.tile([NP, 4], F32, tag="d1l")  # lam * den1
            nc.vector.tensor_scalar(
                d1l[:], u0[:, :, 0], lam_sb[:, 0:1], None, ALU.mult
            )
            # eps applies to normalized x -> deferred ms needs eps*(d1*d2)^2
            dd = ep.tile([NP, 4], F32, tag="dd")  # sqrt(eps)*den1*den2
            nc.vector.scalar_tensor_tensor(
                dd[:], u0[:, :, 0], float(np.sqrt(EPS)), u1[:, :, 0],
                ALU.mult, ALU.mult,
            )
            edd = ep.tile([NP, 4], F32, tag="edd")  # eps*(den1*den2)^2
            nc.vector.tensor_tensor(edd[:], dd[:], dd[:], ALU.mult)
            t2 = ep.tile([NP, 4, E], F32, tag="t2")  # lam*den1*U2
            eng_t2 = nc.gpsimd if epi_pool else nc.vector
            eng_t2.tensor_tensor(
                t2[:], u1[:, :, 1:E + 1],
                d1l[:].unsqueeze(2).broadcast_to([NP, 4, E]),
                ALU.mult,
            )
            for qi in range(4):
                qt_g = 4 * qb + qi
                nc.vector.scalar_tensor_tensor(
                    x_all[:, qt_g, :],
                    u0[:, qi, 1:E + 1],
                    u1[:, qi, 0:1],
                    t2[:, qi, :],
                    ALU.mult, ALU.subtract,
                )
                if epi_act:
                    # ms = mean(x^2) on ACT: Square(x/sqrt(E)) + free-axis accum
                    nc.scalar.activation(
                        ud["o_sb"][:, qt_g, :], x_all[:, qt_g, :], AF.Square,
                        scale=float(E) ** -0.5,
                        accum_out=ms_all[:, qt_g:qt_g + 1],
                    )
                else:
                    xsq = ep.tile([NP, E], F32, tag="xsq")
                    nc.vector.scalar_tensor_tensor(
                        xsq[:], x_all[:, qt_g, :], 1.0 / E, x_all[:, qt_g, :],
                        ALU.mult, ALU.mult,
                    )
                    nc.vector.reduce_sum(
                        ms_all[:, qt_g:qt_g + 1], xsq[:],
                        axis=mybir.AxisListType.X,
                    )
            nc.vector.tensor_tensor(
                ms_all[:, 4 * qb:4 * qb + 4],
                ms_all[:, 4 * qb:4 * qb + 4],
                edd[:], ALU.add,
            )
            if qb_finale:
                lo, hi = 4 * qb, 4 * qb + 4
            elif ud["last"]:
                lo, hi = 0, NKT
            else:
                return
            # ---- finale: rs = 0.2*rsqrt(ms[lo:hi]); out tiles lo..hi -----
            n_t = hi - lo
            ms_sl = ms_all[:, lo:hi]
            sh = ep.tile([NP, n_t], I32, tag="sh")
            nc.vector.tensor_scalar(
                sh[:], ms_sl.bitcast(I32), 1, None, ALU.logical_shift_right
            )
            y = ep.tile([NP, n_t], F32, tag="y")
            nc.vector.tensor_tensor(
                y[:].bitcast(I32),
                magic[:].broadcast_to([NP, n_t]).bitcast(I32),
                sh[:], ALU.subtract,
            )
            mh = ep.tile([NP, n_t], F32, tag="mh")  # 0.5*ms
            nc.vector.tensor_scalar(mh[:], ms_sl, 0.5, None, ALU.mult)
            for it in range(3):
                yy = ep.tile([NP, n_t], F32, tag="yy", name="yy")
                nc.vector.tensor_tensor(yy[:], y[:], y[:], ALU.mult)
                nc.vector.tensor_tensor(yy[:], yy[:], mh[:], ALU.mult)
                fin = -OUT_SCALE if it == 2 else -1.0
                nc.vector.tensor_scalar(
                    yy[:], yy[:], 1.5, fin, ALU.subtract, ALU.mult
                )
                yn = ep.tile([NP, n_t], F32, tag="yn", name="yn")
                nc.vector.tensor_tensor(yn[:], y[:], yy[:], ALU.mult)
                y = yn
            o_sb = ud["o_sb"]
            if epi_act:
                for t in range(lo, hi):
                    nc.scalar.activation(o_sb[:, t, :], x_all[:, t, :],
                                         AF.Copy, scale=y[:, t - lo:t - lo + 1])
            else:
                nc.vector.tensor_tensor(
                    o_sb[:, lo:hi, :], x_all[:, lo:hi, :],
                    y[:].unsqueeze(2).broadcast_to([NP, n_t, E]),
                    ALU.mult,
                )
            if not w_is_ones:
                nc.vector.tensor_tensor(
                    o_sb[:, lo:hi, :], o_sb[:, lo:hi, :],
                    wb_sb[:].unsqueeze(1).broadcast_to([NP, n_t, E]),
                    ALU.mult,
                )
            odma_eng = nc.gpsimd if odma_pool else nc.sync
            odma_eng.dma_start(
                out_d[p].rearrange("(t q) e -> q t e", q=NP)[:, lo:hi, :],
                o_sb[:, lo:hi, :]
            )

        rep_ctx = tc.For_i(0, repeat, 1) if repeat > 1 else None
        if rep_ctx is not None:
            ctx.enter_context(rep_ctx)

        if "compute" in skip:  # DMA-only floor probe
            for p in range(PAIRS):
                qt_sb = io.tile([NP, L], BF16, tag="qt", name="qt")
                kt_sb = io.tile([NP, L], BF16, tag="kt", name="kt")
                vv_sb = io.tile([NP, NKT, E + 1], BF16, tag="vv", name="vv")
                nc.sync.dma_start(qt_sb[:], qt_d[p])
                nc.sync.dma_start(kt_sb[:], kt_d[p])
                nc.sync.dma_start(
                    vv_sb[:], vv_d[p].rearrange("(t k) e -> k t e", k=NP))
                o_sb = xp.tile([NP, NKT, E], F32, tag="o", name="o")
                nc.gpsimd.memset(o_sb[:, 0, 0:1], 0.0)
                nc.vector.tensor_tensor(
                    o_sb[:, 0, 0:1], qt_sb[:, 0:2].bitcast(F32),
                    kt_sb[:, 0:2].bitcast(F32), ALU.mult)
                nc.vector.tensor_copy(o_sb[:, 0, 2:4], vv_sb[:, 0, 0:2])
                nc.sync.dma_start(
                    out_d[p].rearrange("(t q) e -> q t e", q=NP), o_sb[:])
            units = []
        else:
            units = []
            for p in range(PAIRS):
                qbs = list(range(NQB))
                if tail_reorder and p == PAIRS - 1:
                    qbs = qbs[::-1]
                for i_qb, qb in enumerate(qbs):
                    for s in range(2):
                        units.append((p, qb, s, i_qb == 0 and s == 0,
                                      i_qb == NQB - 1 and s == 1))

        pending = None  # previous unit: PV not yet emitted
        pair_state = {}
        pvq = []  # rolling queue of (unit, stream-item, is_last)

        def drain_pvq(n):
            for _ in range(n):
                if not pvq:
                    return
                ud_i, it, last = pvq.pop(0)
                emit_stream(ud_i, [it])
                if last:
                    finish_unit(ud_i)
        for p, qb, s, first_of_pair, last_of_pair in units:
                    if first_of_pair:
                        qt_sb = io.tile([NP, L], BF16, tag="qt", name="qt")
                        kt_sb = io.tile([NP, L], BF16, tag="kt", name="kt")
                        vv_sb = io.tile([NP, NKT, E + 1], BF16, tag="vv",
                                        name="vv")
                        nc.sync.dma_start(qt_sb[:], qt_d[p])
                        nc.sync.dma_start(kt_sb[:], kt_d[p])
                        nc.sync.dma_start(
                            vv_sb[:],
                            vv_d[p].rearrange("(t k) e -> k t e", k=NP))
                        x_all = xp.tile([NP, NKT, E], F32, tag="x", name="x")
                        ms_all = xp.tile([NP, NKT], F32, tag="ms", name="ms")
                        o_sb = xp.tile([NP, NKT, E], F32, tag="o", name="o")
                        pair_state = dict(qt=qt_sb, kt=kt_sb, vv=vv_sb,
                                          x_all=x_all, ms_all=ms_all, o_sb=o_sb)
                    nki = 4 * qb + 4
                    ud = dict(p=p, qb=qb, s=s, nki=nki, last=last_of_pair,
                              **pair_state)
                    ud["pt"] = ptp.tile([NP, nki, QB], BF16, tag=f"pt{s}",
                                        name=f"pt{s}")
                    if "pv" not in skip and "epi" not in skip:
                        ud["u_sb"] = usb.tile([NP, 4, E + 1], F32,
                                              tag=f"u{s}", name=f"u{s}")
                    if s == 1:
                        ud["peer"] = prev_unit  # the s=0 unit of same (p, qb)

                    pushed = 0
                    if pending is not None:
                        st_items = build_stream(pending)
                        for idx, it in enumerate(st_items):
                            pvq.append((pending, it,
                                        idx == len(st_items) - 1))
                        pushed = len(st_items)
                    nchunks = nki // CHUNK
                    if interleave:
                        keep = pushed // 2 if lag2 else 0
                        target = max(0, len(pvq) - keep)
                        nb = nchunks if fine_batch else max(1, nchunks - 1)
                        bsz = (target + nb - 1) // nb if target else 0
                        done = 0
                        for c in range(nchunks):
                            if pv_first_slot:
                                if (fine_batch or c >= 1) and done < target:
                                    take = min(bsz, target - done)
                                    drain_pvq(take)
                                    done += take
                                emit_s_chunk(ud, c)
                            else:
                                emit_s_chunk(ud, c)
                                if (fine_batch or c >= 1) and done < target:
                                    take = min(bsz, target - done)
                                    drain_pvq(take)
                                    done += take
                        if done < target:
                            drain_pvq(target - done)
                    else:
                        drain_pvq(len(pvq))
                        for c in range(nchunks):
                            emit_s_chunk(ud, c)
                    prev_unit = ud
                    pending = ud
        # drain
        if pending is not None:
            st_items = build_stream(pending)
            for idx, it in enumerate(st_items):
                pvq.append((pending, it, idx == len(st_items) - 1))
        drain_pvq(len(pvq))

    nc.compile()
    return nc


_PROGRAM_CACHE: dict = {}


def _get_program(w_is_ones: bool, repeat: int = 1) -> bass.Bass:
    key = (w_is_ones, repeat)
    if key not in _PROGRAM_CACHE:
        _PROGRAM_CACHE[key] = _build_program(w_is_ones, repeat)
    return _PROGRAM_CACHE[key]


def make_in_maps(query, key, value, lambda_q1, lambda_k1, lambda_q2, lambda_k2,
                 sub_norm_w):
    """Host-side shard/pack. Returns (in_maps, w_is_ones)."""
    query = np.asarray(query, dtype=np.float32)
    key = np.asarray(key, dtype=np.float32)
    value = np.asarray(value, dtype=np.float32)
    lam = float(
        np.exp(np.sum(np.float64(lambda_q1) * np.float64(lambda_k1)))
        - np.exp(np.sum(np.float64(lambda_q2) * np.float64(lambda_k2)))
        + LAMBDA_INIT
    )
    w = np.asarray(sub_norm_w, dtype=np.float32)
    w_is_ones = bool(np.all(w == 1.0))

    import ml_dtypes

    bf16 = ml_dtypes.bfloat16
    q5 = query.reshape(B, L, H, 2 * D)
    k5 = key.reshape(B, L, H, 2 * D)
    v4 = value.reshape(B, L, H, E)
    lam_arr = np.full((NP, 1), lam, dtype=np.float32)
    wb = np.broadcast_to(w[None, :], (NP, E)).copy() if not w_is_ones else None

    in_maps = []
    for c in range(N_CORES):
        qt = np.empty((PAIRS, NP, L), dtype=bf16)
        kt = np.empty((PAIRS, NP, L), dtype=bf16)
        vv = np.empty((PAIRS, L, E + 1), dtype=bf16)
        for p in range(PAIRS):
            f = c * PAIRS + p
            b, h = divmod(f, H)
            qt[p] = q5[b, :, h].T.astype(bf16)
            kt[p] = k5[b, :, h].T.astype(bf16)
            vv[p, :, 0] = 1.0
            vv[p, :, 1:] = v4[b, :, h].astype(bf16)
        m = {"qt": qt, "kt": kt, "vv": vv, "lam": lam_arr}
        if not w_is_ones:
            m["wb"] = wb
        in_maps.append(m)
    return in_maps, w_is_ones


def assemble_output(results) -> np.ndarray:
    out = np.empty((B, L, H * E), dtype=np.float32)
    for c in range(N_CORES):
        o = results[c]["out"]
        for p in range(PAIRS):
            f = c * PAIRS + p
            b, h = divmod(f, H)
            out[b, :, h * E:(h + 1) * E] = o[p]
    return out


def kernel(query, key, value, lambda_q1, lambda_k1, lambda_q2, lambda_k2,
           sub_norm_w, **_unused):
    in_maps, w_is_ones = make_in_maps(
        query, key, value, lambda_q1, lambda_k1, lambda_q2, lambda_k2, sub_norm_w
    )
    nc = _get_program(w_is_ones)
    res = run_bass_kernel_spmd(nc, in_maps, core_ids=list(range(N_CORES)))
    return assemble_output(res.results)
